# revision 28
# baseline (speedup 1.0000x reference)
"""Multi-head self-attention (b=2, n=2048, emb=1024, heads=16) on 8 trn2 cores.

Sharding: core c = (b, hg) with b = c // 4, hg = c % 4. Data parallel over
batch, tensor parallel over head-groups (4 heads / 256 emb-cols per core).
Each core computes Q/K/V projections for its heads, full attention for its
heads, and a partial output projection ctx_hg @ Wo[:, hg_slice].T of shape
[2048, 1024] (fp16). The host sums the 4 partials per batch and adds the
rank-1 bias term bv @ Wo.T + bo.

v2 redesign (ACT-bound target ~150-170us):
- nq attention chunk = 512 (4 j-chunks). S^T per (t, head-pair) computed as
  TWO row-tiled 64-contraction matmuls at tile_position (0,0)/(64,0) that
  run CONCURRENTLY on the PE (heads of a pair live in partitions 0-63 /
  64-127 of qT/kT), each writing one full PSUM bank [128, 512] f32.
- exp runs 1024-wide over the 2-bank S tile (both heads at once); the exp
  stream is the critical resource (128 x ~1.15us = 147us) so everything
  else is paced to hide under it.
- ctx^T accumulates [65, 512] per head over 16 nk-chunks; ones-column is
  FIRST (row 0 = softmax denominator, lands on physical partition 0 so
  gpsimd.partition_broadcast needs no staging copy). Normalize =
  copy (frees bank) + reciprocal_approx_fast on [1,512] + broadcast +
  one scalar_tensor_tensor multiply into fp16 ctxT.
- x is loaded ONCE (32KB/partition resident) and shared by both pairs'
  projections; V projection computed once for all 4 heads with a 256-wide
  moving operand.
- All projection/out-projection work is split into <=600ns parcels and
  deadline-scheduled into the per-t ACT slack so the PE never blocks the
  exp stream; j=0 of pair 0 interleaves K/V/Q chunk-wise with the S/exp
  stream as x chunks arrive from HBM.
- output partials stored/DMA'd as fp16.
"""

import os
import sys

for _p in ("/opt/trn_rl_repo", "/root/.axon_site/_ro/trn_rl_repo"):
    if os.path.isdir(_p) and _p not in sys.path:
        sys.path.append(_p)

import numpy as np

import concourse.bass as bass  # noqa: F401
import concourse.mybir as mybir
import concourse.tile as tile
from concourse import bacc
from concourse.bass_utils import run_bass_kernel_spmd

B, N, EMB, HEADS, HD = 2, 2048, 1024, 16, 64
N_CORES = 8
TP = 4                      # head-group shards per batch
DQ = EMB // TP              # 256 emb-cols (4 heads) per core
SCALE = HD ** -0.5          # 0.125

F32 = mybir.dt.float32
F16 = mybir.dt.float16
FP = mybir.ActivationFunctionType

NQ = 512                    # token chunk for projections and attention
NJ = N // NQ                # 4 chunks
NKC = 128                   # nk chunk (ctx contraction)
NT = N // NKC               # 16 nk chunks
KC = EMB // 128             # 8 contraction chunks for projections


def build_program():
    nc = bacc.Bacc("TRN2", target_bir_lowering=False, debug=False,
                   num_devices=N_CORES)

    xT = nc.dram_tensor("xT", [EMB, N], F16, kind="ExternalInput").ap()
    wqT = nc.dram_tensor("wqT", [EMB, DQ], F16, kind="ExternalInput").ap()
    wkT = nc.dram_tensor("wkT", [EMB, DQ], F16, kind="ExternalInput").ap()
    wvT = nc.dram_tensor("wvT", [EMB, DQ], F16, kind="ExternalInput").ap()
    woT = nc.dram_tensor("woT", [DQ, EMB], F16, kind="ExternalInput").ap()
    bqd = nc.dram_tensor("bq_s", [DQ], F32, kind="ExternalInput").ap()
    bkd = nc.dram_tensor("bk_s", [DQ], F32, kind="ExternalInput").ap()
    out_part = nc.dram_tensor("out_part", [N, EMB], F16,
                              kind="ExternalOutput").ap()

    add, mult = mybir.AluOpType.add, mybir.AluOpType.mult
    bypass = mybir.AluOpType.bypass

    with tile.TileContext(nc) as tc:
        with (
            tc.tile_pool(name="const", bufs=1) as const,
            tc.tile_pool(name="persist", bufs=1) as persist,
            tc.tile_pool(name="epool", bufs=14) as epool,
            tc.tile_pool(name="npool", bufs=2) as npool,
            tc.tile_pool(name="opool", bufs=NT) as opool,
            tc.tile_pool(name="o16pool", bufs=4) as o16pool,
            # PSUM budget (8 banks): s 2x2 + c 1+1 + pp 1x2 = 8
            tc.tile_pool(name="spool", bufs=2, space="PSUM") as spool,
            tc.tile_pool(name="cpool", bufs=1, space="PSUM") as cpool,
            tc.tile_pool(name="ppool", bufs=2, space="PSUM") as ppool,
        ):
            # ---- early exp table load (~2.7us) under the DMA prefix ----
            zt = const.tile([1, 1], F32, tag="zt", name="zt")
            nc.vector.memset(zt, 0.0)
            zo = const.tile([1, 1], F32, tag="zo", name="zo")
            nc.scalar.activation(zo, zt, FP.Exp)


            # ---- weights ----
            # DMA order matters at startup: the first K-projection group
            # needs wk[k] + x(0)[k] only, so those two streams interleave
            # per-k; wq/x(1), then wv/biases follow.
            wq_sb = const.tile([128, KC, DQ], F16, tag="wq", name="wq")
            wk_sb = const.tile([128, KC, DQ], F16, tag="wk", name="wk")
            wv_sb = const.tile([128, KC, DQ], F16, tag="wv", name="wv")
            wo_sb = const.tile([128, 2, EMB], F16, tag="wo", name="wo")  # deferred DMA
            bq_sb = const.tile([128, 2], F32, tag="bq", name="bq")
            bk_sb = const.tile([128, 2], F32, tag="bk", name="bk")

            # ---- persistent activations ----
            x_all = persist.tile([128, KC, NJ, NQ], F16, tag="x", name="x")
            qT = [persist.tile([128, N], F16, tag=f"qT{p}", name=f"qT{p}") for p in range(2)]
            kT = [persist.tile([128, N], F16, tag=f"kT{p}", name=f"kT{p}") for p in range(2)]
            ctxT = [persist.tile([128, N], F16, tag=f"ctxT{p}", name=f"ctxT{p}") for p in range(2)]
            # V for 4 local heads: [nk-part, t, head*65 + (0:64 | ones)]
            v_all = persist.tile([128, NT, 4 * (HD + 1)], F16, tag="v", name="v")
            for h in range(4):
                nc.vector.memset(v_all[:, :, h * 65 + 64], 1.0)

            def x_dma(n):
                # n=0,2: per-k on the (startup-idle) GpSimd queue so the
                # first K-projection groups start on partial arrivals;
                # n=1,3: one monolithic 3D DMA each on the Sync queue,
                # pipelined behind the weights.
                if n % 2 == 0:
                    for k in range(KC):
                        nc.gpsimd.dma_start(
                            out=x_all[:, k, n, :],
                            in_=xT[k * 128:(k + 1) * 128, n * NQ:(n + 1) * NQ])
                else:
                    nc.sync.dma_start(
                        out=x_all[:, :, n, :],
                        in_=xT.rearrange("(k p) t -> p k t", p=128)[
                            :, :, n * NQ:(n + 1) * NQ])

            # ---- projection parcels ----
            # kq_group(p, n): 8 accumulating MMs + bias-add, emitted as
            # 4x(2 MMs) + 1 DVE parcel so each slots into per-t ACT slack.
            _pp = {}

            def kq_mms(p, n, wsb, key, ks):
                if ks == 0:
                    _pp[key] = ppool.tile([128, NQ], F32, tag="pp", name="pp")
                ps = _pp[key]
                for k in (ks, ks + 1):
                    nc.tensor.matmul(
                        ps, wsb[:, k, p * 128:(p + 1) * 128],
                        x_all[:, k, n, :], start=(k == 0), stop=(k == KC - 1))

            def kq_fin(p, n, bsb, dst, key):
                ps = _pp.pop(key)
                nc.vector.tensor_tensor(
                    out=dst[p][:, n * NQ:(n + 1) * NQ], in0=ps,
                    in1=bsb[:, p:p + 1].broadcast_to([128, NQ]), op=add)

            def kq_parcels(p, n, wsb, bsb, dst, key):
                out = [lambda ks=ks: kq_mms(p, n, wsb, key, ks)
                       for ks in (0, 2, 4, 6)]
                out.append(lambda: kq_fin(p, n, bsb, dst, key))
                return out

            def kq_pair_mms(p, n0, n1, wsb, key, ks):
                # two n-chunks accumulate in parallel pp banks; each weight
                # chunk is loaded once and streams both moving operands.
                if ks == 0:
                    _pp[(key, 0)] = ppool.tile([128, NQ], F32, tag="pp",
                                               name="pp")
                    _pp[(key, 1)] = ppool.tile([128, NQ], F32, tag="pp",
                                               name="pp")
                for k in (ks, ks + 1):
                    for i, n in enumerate((n0, n1)):
                        nc.tensor.matmul(
                            _pp[(key, i)], wsb[:, k, p * 128:(p + 1) * 128],
                            x_all[:, k, n, :], start=(k == 0),
                            stop=(k == KC - 1))

            def kq_pair_parcels(p, n0, n1, wsb, bsb, dst, key):
                out = [lambda ks=ks: kq_pair_mms(p, n0, n1, wsb, key, ks)
                       for ks in (0, 2, 4, 6)]

                def fin():
                    for i, n in enumerate((n0, n1)):
                        ps = _pp.pop((key, i))
                        nc.vector.tensor_tensor(
                            out=dst[p][:, n * NQ:(n + 1) * NQ], in0=ps,
                            in1=bsb[:, p:p + 1].broadcast_to([128, NQ]),
                            op=add)
                out.append(fin)
                return out

            def v_mms(n, tl, key, ks):
                if ks == 0:
                    _pp[key] = ppool.tile([128, NQ], F32, tag="pp", name="pp")
                ps = _pp[key]
                for k in (ks, ks + 1):
                    nc.tensor.matmul(
                        ps[:, 0:256], x_all[:, k, n, tl * 128:(tl + 1) * 128],
                        wv_sb[:, k, :], start=(k == 0), stop=(k == KC - 1))

            def v_fin(n, tl, key):
                ps = _pp.pop(key)
                t = n * 4 + tl
                vv = v_all[:, t, :].rearrange("p (h c) -> p h c", c=65)
                nc.vector.tensor_copy(
                    out=vv[:, :, 0:64],
                    in_=ps[:, 0:256].rearrange("p (h c) -> p h c", c=64))

            def v_parcels(n, tl):
                key = ("v", n, tl)
                out = [lambda ks=ks: v_mms(n, tl, key, ks)
                       for ks in (0, 2, 4, 6)]
                out.append(lambda: v_fin(n, tl, key))
                return out

            # ---- out-projection parcels ----
            o_tiles = {}

            def out_proj_parcel(kp, m):
                # both eo halves in one parcel: the stationary ctxT chunk is
                # loaded once and serves two 512-wide matmuls.
                if kp == 0:
                    o_tiles[m] = opool.tile([128, EMB], F32, tag="o", name="o")
                o = o_tiles[m]
                pos = []
                for eo in range(2):
                    po = ppool.tile([128, NQ], F32, tag="pp", name="pp")
                    nc.tensor.matmul(
                        po, ctxT[kp][:, m * 128:(m + 1) * 128],
                        wo_sb[:, kp, eo * NQ:(eo + 1) * NQ],
                        start=True, stop=True)
                    pos.append(po)
                for eo in range(2):
                    if kp == 0:
                        nc.vector.tensor_copy(o[:, eo * NQ:(eo + 1) * NQ],
                                              pos[eo])
                    else:
                        o16 = o_tiles[("f", m)]
                        nc.vector.tensor_tensor(
                            out=o16[:, eo * NQ:(eo + 1) * NQ],
                            in0=o[:, eo * NQ:(eo + 1) * NQ], in1=pos[eo],
                            op=add)

            def out_proj_p1(m):
                o_tiles[("f", m)] = o16pool.tile([128, EMB], F16, tag="o16",
                                                 name="o16")
                out_proj_parcel(1, m)
                o16 = o_tiles.pop(("f", m))
                o_tiles.pop(m)
                for eo in range(2):
                    nc.sync.dma_start(
                        out=out_part[m * 128:(m + 1) * 128,
                                     eo * NQ:(eo + 1) * NQ],
                        in_=o16[:, eo * NQ:(eo + 1) * NQ])

            # ---- attention machinery ----
            def s_pair(p, j, t):
                sg = spool.tile([128, 2, NQ], F32, tag="s", name="s")
                for h in range(2):
                    nc.tensor.matmul(
                        sg[:, h, :],
                        kT[p][64 * h:64 * h + 64, t * 128:(t + 1) * 128],
                        qT[p][64 * h:64 * h + 64, j * NQ:(j + 1) * NQ],
                        start=True, stop=True)
                return sg

            def exp_act(sg):
                e = epool.tile([128, 2, NQ], F16, tag="e", name="e")
                nc.scalar.activation(e, sg, FP.Exp, scale=SCALE)
                return e

            def ctx_pair(p, cps, e, t):
                for h in range(2):
                    hl = 2 * p + h
                    nc.tensor.matmul(
                        cps[h], v_all[:, t, hl * 65:(hl + 1) * 65],
                        e[:, h, :], start=(t == 0), stop=(t == NT - 1))

            def normalize(p, j, h, cps):
                # copy out of PSUM first (frees the ctx bank for next j);
                # rowsum sits in row 64 -> stage to partition 0 for the
                # gpsimd broadcast, reciprocal on the [1, 512] staged row.
                cs = npool.tile([65, NQ], F32, tag="cs", name="cs")
                nc.vector.tensor_copy(cs, cps[h])
                rs = npool.tile([1, NQ], F32, tag="rs", name="rs")
                nc.vector.tensor_copy(rs, cs[64:65, :])
                rc = npool.tile([1, NQ], F32, tag="rc", name="rc")
                nc.vector.reciprocal_approx_fast(rc, rs)
                rb = npool.tile([64, NQ], F32, tag="rb", name="rb")
                nc.gpsimd.partition_broadcast(rb, rc)
                nc.vector.scalar_tensor_tensor(
                    out=ctxT[p][h * 64:(h + 1) * 64, j * NQ:(j + 1) * NQ],
                    in0=cs[0:64, :], scalar=1.0, in1=rb,
                    op0=mult, op1=mult)

            # ---- schedule ----
            # One flat software pipeline over all 128 (p, j, t) iterations:
            # iteration g emits S-pair(g), exp(g), then ctx-pair(g-2) (lag 2
            # so ctx never waits on the exp semaphore), then filler parcels
            # from sched[g]. j/p boundaries are crossed seamlessly; the
            # normalize for j is emitted right after its last ctx pair.
            sched = {}

            def put(g, f):
                sched.setdefault(g, []).append(f)

            cps_by = {}
            pend = []

            def emit_ctx(g2):
                p2, r2 = divmod(g2, 64)
                j2, t2 = divmod(r2, 16)
                if t2 == 0:
                    cps_by[(p2, j2)] = [
                        cpool.tile([65, NQ], F32, tag=f"c{h}", name=f"c{h}")
                        for h in range(2)]
                e2 = pend.pop(0)
                ctx_pair(p2, cps_by[(p2, j2)], e2, t2)
                if t2 == NT - 1:
                    cps2 = cps_by.pop((p2, j2))
                    for h in range(2):
                        normalize(p2, j2, h, cps2)

            # ---------- prefix ----------
            # Sync queue: [biases, wk, wv, wq, x1, x3]; GpSimd queue: x0, x2
            # per-k. The first K group starts at wk + x0[0]; Q0(0) (the first
            # S dependency) completes right after wq lands.
            nc.sync.dma_start(out=bk_sb, in_=bkd.rearrange("(m p) -> p m", p=128))
            nc.sync.dma_start(out=bq_sb, in_=bqd.rearrange("(m p) -> p m", p=128))
            # pair-0 halves of wk/wq first: Q0(0) (the first S dependency)
            # completes as soon as wq[:,:,0:128] lands; pair-1 halves are
            # deferred into the filler schedule (needed from slot ~17).
            nc.sync.dma_start(out=wk_sb[:, :, 0:128], in_=wkT.rearrange(
                "(k p) d -> p k d", p=128)[:, :, 0:128])
            nc.sync.dma_start(out=wq_sb[:, :, 0:128], in_=wqT.rearrange(
                "(k p) d -> p k d", p=128)[:, :, 0:128])
            x_dma(0)
            x_dma(2)
            nc.sync.dma_start(out=x_all[:, :, 1, :],
                              in_=xT.rearrange("(k p) t -> p k t", p=128)[
                                  :, :, NQ:2 * NQ])
            nc.sync.dma_start(out=wv_sb, in_=wvT.rearrange(
                "(k p) d -> p k d", p=128))
            nc.sync.dma_start(out=x_all[:, :, 3, :],
                              in_=xT.rearrange("(k p) t -> p k t", p=128)[
                                  :, :, 3 * NQ:4 * NQ])
            for f in kq_parcels(0, 0, wk_sb, bk_sb, kT, ("k0", 0)):
                f()
            for f in kq_parcels(0, 0, wq_sb, bq_sb, qT, ("q0", 0)):
                f()

            # ---------- filler schedule ----------
            # pair-0 window (g 0..63):
            #   V(n, tl) whole-group at g=4n+tl+1 (ctx(t) runs at g=t+2)
            #   K0(n) parcels at g=4n-4..4n-1; Q0(n) parcels at g=16n-5..
            for n in range(NJ):
                for tl in range(4):
                    for i, f in enumerate(v_parcels(n, tl)):
                        put(4 * n + tl + 2 + i, f)
            for n in (1, 2, 3):
                for i, f in enumerate(kq_parcels(0, n, wk_sb, bk_sb, kT,
                                                 ("k0", n))):
                    put(4 * n - 4 + min(i, 3), f)
                for i, f in enumerate(kq_parcels(0, n, wq_sb, bq_sb, qT,
                                                 ("q0", n))):
                    put(16 * n - 5 + i, f)
            put(6, lambda: nc.sync.dma_start(
                out=wk_sb[:, :, 128:256], in_=wkT.rearrange(
                    "(k p) d -> p k d", p=128)[:, :, 128:256]))
            put(8, lambda: nc.sync.dma_start(
                out=wq_sb[:, :, 128:256], in_=wqT.rearrange(
                    "(k p) d -> p k d", p=128)[:, :, 128:256]))
            put(13, lambda: nc.sync.dma_start(
                out=wo_sb, in_=woT.rearrange("(k p) e -> p k e", p=128)))
            free_iters = [g for g in range(20, 64)
                          if len(sched.get(g, [])) == 0]
            fill = []
            fill += kq_pair_parcels(1, 0, 1, wk_sb, bk_sb, kT, ("k1", 0))
            fill += kq_pair_parcels(1, 0, 1, wq_sb, bq_sb, qT, ("q1", 0))
            fill += kq_pair_parcels(1, 2, 3, wk_sb, bk_sb, kT, ("k1", 2))
            fill += kq_pair_parcels(1, 2, 3, wq_sb, bq_sb, qT, ("q1", 2))
            for i, f in enumerate(fill):
                put(free_iters[i * len(free_iters) // len(fill)], f)
            # pair-1 window (g 64..127): out-proj pass 0 at g=64..95;
            # pass 1 for j at g=64+16(j+1)+2.. (8 parcels); j=3 in the tail.
            for m in range(NT):
                put(64 + 2 * m, lambda m=m: out_proj_parcel(0, m))
            for j in range(NJ - 1):
                for i, m in enumerate(range(4 * j, 4 * j + 4)):
                    put(64 + 16 * (j + 1) + 2 + 3 * i,
                        lambda m=m: out_proj_p1(m))

            # ---------- main pipeline (batches of 2 t-iterations) ----------
            # [S-pair(g), S-pair(g+1)] then [ctx(g-2), ctx(g-1)] so the PE
            # switches between 64-row-tile mode and full-array mode once per
            # batch instead of twice per iteration (each switch exposes the
            # ~150ns pipe drain to the next matmul).
            # ctx lag starts at 8 (j=0's V projections spread over the
            # first ~24 slots) and tightens to 2 by mid-j=1 so the drain
            # after the last exp stays short.
            next_g2 = 0
            for gb in range(0, 128, 2):
                sgs = []
                for g in (gb, gb + 1):
                    p, r = divmod(g, 64)
                    j, t = divmod(r, 16)
                    sgs.append(s_pair(p, j, t))
                for sg in sgs:
                    pend.append(exp_act(sg))
                for g in (gb, gb + 1):
                    for f in sched.get(g, ()):
                        f()
                lag = 8 if gb < 32 else max(2, 8 - (gb - 32) // 4)
                while len(pend) > lag:
                    emit_ctx(next_g2)
                    next_g2 += 1

            # ---------- tail: ctx/normalize for (p1, j3, t14..15) ----------
            # then pipelined per-m normalize-chunk + out-proj + DMA.
            p3, j3 = 1, NJ - 1
            cps3 = None
            last_e = None
            for g2 in (126, 127):
                t2 = g2 % 16
                e2 = pend.pop(0)
                ctx_pair(p3, cps_by[(p3, j3)], e2, t2)
                last_e = e2
            cps3 = cps_by.pop((p3, j3))
            css, rbs = [], []
            for h in range(2):
                cs = npool.tile([65, NQ], F32, tag="cs", name="cs")
                nc.vector.tensor_copy(cs, cps3[h])
                rs = npool.tile([1, NQ], F32, tag="rs", name="rs")
                nc.vector.tensor_copy(rs, cs[64:65, :])
                rc = npool.tile([1, NQ], F32, tag="rc", name="rc")
                nc.vector.reciprocal_approx_fast(rc, rs)
                rb = npool.tile([64, NQ], F32, tag="rb", name="rb")
                nc.gpsimd.partition_broadcast(rb, rc)
                css.append(cs)
                rbs.append(rb)
            for m in range(4 * j3, 4 * j3 + 4):
                mo = (m - 4 * j3) * 128
                for h in range(2):
                    nc.vector.scalar_tensor_tensor(
                        out=ctxT[p3][h * 64:(h + 1) * 64,
                                     j3 * NQ + mo:j3 * NQ + mo + 128],
                        in0=css[h][0:64, mo:mo + 128], scalar=1.0,
                        in1=rbs[h][:, mo:mo + 128], op0=mult, op1=mult)
                out_proj_p1(m)

    nc.compile()
    return nc


_NC_CACHE = {}


def _get_program():
    if "nc" not in _NC_CACHE:
        _NC_CACHE["nc"] = build_program()
    return _NC_CACHE["nc"]


def make_in_maps(x, Wq, bq, Wk, bk, Wv, bv, Wo, bo):
    x = np.asarray(x)
    xTs = [np.ascontiguousarray(x[b].T.astype(np.float16)) for b in range(B)]
    in_maps = []
    for c in range(N_CORES):
        b, hg = divmod(c, TP)
        sl = slice(hg * DQ, (hg + 1) * DQ)
        in_maps.append({
            "xT": xTs[b],
            "wqT": np.ascontiguousarray(np.asarray(Wq, np.float16)[sl, :].T),
            "wkT": np.ascontiguousarray(np.asarray(Wk, np.float16)[sl, :].T),
            "wvT": np.ascontiguousarray(np.asarray(Wv, np.float16)[sl, :].T),
            "woT": np.ascontiguousarray(np.asarray(Wo, np.float16)[:, sl].T),
            "bq_s": np.ascontiguousarray(np.asarray(bq, np.float32)[sl]),
            "bk_s": np.ascontiguousarray(np.asarray(bk, np.float32)[sl]),
        })
    return in_maps


def assemble_output(results, Wv_bias_term):
    out = np.empty((B, N, EMB), np.float32)
    for b in range(B):
        acc = results[b * TP]["out_part"].astype(np.float32)
        for g in range(1, TP):
            acc = acc + results[b * TP + g]["out_part"].astype(np.float32)
        out[b] = acc + Wv_bias_term
    return out


def kernel(x, Wq, bq, Wk, bk, Wv, bv, Wo, bo):
    nc = _get_program()
    in_maps = make_in_maps(x, Wq, bq, Wk, bk, Wv, bv, Wo, bo)
    res = run_bass_kernel_spmd(nc, in_maps, list(range(N_CORES)))
    bias_term = (np.asarray(bv, np.float32) @ np.asarray(Wo, np.float32).T
                 + np.asarray(bo, np.float32))
    return assemble_output(res.results, bias_term)


# revision 29
# speedup vs baseline: 1.1829x; 1.1829x over previous
"""Multi-head self-attention (b=2, n=2048, emb=1024, heads=16) on 8 trn2 cores.

Sharding: core c = (b, hg) with b = c // 4, hg = c % 4. Data parallel over
batch, tensor parallel over head-groups (4 heads / 256 emb-cols per core).
Each core computes Q/K/V projections for its heads, full attention for its
heads, and a partial output projection ctx_hg @ Wo[:, hg_slice].T of shape
[2048, 1024] (fp16). The host sums the 4 partials per batch and adds the
rank-1 bias term bv @ Wo.T + bo.

v2 redesign (ACT-bound target ~150-170us):
- nq attention chunk = 512 (4 j-chunks). S^T per (t, head-pair) computed as
  TWO row-tiled 64-contraction matmuls at tile_position (0,0)/(64,0) that
  run CONCURRENTLY on the PE (heads of a pair live in partitions 0-63 /
  64-127 of qT/kT), each writing one full PSUM bank [128, 512] f32.
- exp runs 1024-wide over the 2-bank S tile (both heads at once); the exp
  stream is the critical resource (128 x ~1.15us = 147us) so everything
  else is paced to hide under it.
- ctx^T accumulates [65, 512] per head over 16 nk-chunks; ones-column is
  FIRST (row 0 = softmax denominator, lands on physical partition 0 so
  gpsimd.partition_broadcast needs no staging copy). Normalize =
  copy (frees bank) + reciprocal_approx_fast on [1,512] + broadcast +
  one scalar_tensor_tensor multiply into fp16 ctxT.
- x is loaded ONCE (32KB/partition resident) and shared by both pairs'
  projections; V projection computed once for all 4 heads with a 256-wide
  moving operand.
- All projection/out-projection work is split into <=600ns parcels and
  deadline-scheduled into the per-t ACT slack so the PE never blocks the
  exp stream; j=0 of pair 0 interleaves K/V/Q chunk-wise with the S/exp
  stream as x chunks arrive from HBM.
- output partials stored/DMA'd as fp16.
"""

import os
import sys

for _p in ("/opt/trn_rl_repo", "/root/.axon_site/_ro/trn_rl_repo"):
    if os.path.isdir(_p) and _p not in sys.path:
        sys.path.append(_p)

import numpy as np

import concourse.bass as bass  # noqa: F401
import concourse.mybir as mybir
import concourse.tile as tile
from concourse import bacc
from concourse.bass_utils import run_bass_kernel_spmd

B, N, EMB, HEADS, HD = 2, 2048, 1024, 16, 64
N_CORES = 8
TP = 4                      # head-group shards per batch
DQ = EMB // TP              # 256 emb-cols (4 heads) per core
SCALE = HD ** -0.5          # 0.125

F32 = mybir.dt.float32
F16 = mybir.dt.float16
FP = mybir.ActivationFunctionType

NQ = 512                    # token chunk for projections and attention
NJ = N // NQ                # 4 chunks
NKC = 128                   # nk chunk (ctx contraction)
NT = N // NKC               # 16 nk chunks
KC = EMB // 128             # 8 contraction chunks for projections


def build_program():
    nc = bacc.Bacc("TRN2", target_bir_lowering=False, debug=False,
                   num_devices=N_CORES)

    xT = nc.dram_tensor("xT", [EMB, N], F16, kind="ExternalInput").ap()
    wqT = nc.dram_tensor("wqT", [EMB, DQ], F16, kind="ExternalInput").ap()
    wkT = nc.dram_tensor("wkT", [EMB, DQ], F16, kind="ExternalInput").ap()
    wvT = nc.dram_tensor("wvT", [EMB, DQ], F16, kind="ExternalInput").ap()
    woT = nc.dram_tensor("woT", [DQ, EMB], F16, kind="ExternalInput").ap()
    bqd = nc.dram_tensor("bq_s", [DQ], F32, kind="ExternalInput").ap()
    bkd = nc.dram_tensor("bk_s", [DQ], F32, kind="ExternalInput").ap()
    out_part = nc.dram_tensor("out_part", [N, EMB], F16,
                              kind="ExternalOutput").ap()

    add, mult = mybir.AluOpType.add, mybir.AluOpType.mult
    bypass = mybir.AluOpType.bypass

    with tile.TileContext(nc) as tc:
        with (
            tc.tile_pool(name="const", bufs=1) as const,
            tc.tile_pool(name="persist", bufs=1) as persist,
            tc.tile_pool(name="epool", bufs=12) as epool,
            tc.tile_pool(name="npool", bufs=2) as npool,
            tc.tile_pool(name="opool", bufs=NT) as opool,
            tc.tile_pool(name="o16pool", bufs=4) as o16pool,
            # PSUM budget (8 banks): s 2x2 + c 1+1 + pp 1x2 = 8
            tc.tile_pool(name="spool", bufs=2, space="PSUM") as spool,
            tc.tile_pool(name="cpool", bufs=1, space="PSUM") as cpool,
            tc.tile_pool(name="ppool", bufs=2, space="PSUM") as ppool,
        ):
            # ---- early exp table load (~2.7us) under the DMA prefix ----
            zt = const.tile([1, 1], F32, tag="zt", name="zt")
            nc.vector.memset(zt, 0.0)
            zo = const.tile([1, 1], F32, tag="zo", name="zo")
            nc.scalar.activation(zo, zt, FP.Exp)


            # ---- weights ----
            # DMA order matters at startup: the first K-projection group
            # needs wk[k] + x(0)[k] only, so those two streams interleave
            # per-k; wq/x(1), then wv/biases follow.
            wq_sb = const.tile([128, KC, DQ], F16, tag="wq", name="wq")
            wk_sb = const.tile([128, KC, DQ], F16, tag="wk", name="wk")
            wv_sb = const.tile([128, KC, DQ], F16, tag="wv", name="wv")
            wo_sb = const.tile([128, 2, EMB], F16, tag="wo", name="wo")  # deferred DMA
            bq_sb = const.tile([128, 2], F32, tag="bq", name="bq")
            bk_sb = const.tile([128, 2], F32, tag="bk", name="bk")

            # ---- persistent activations ----
            x_all = persist.tile([128, KC, NJ, NQ], F16, tag="x", name="x")
            qT = [persist.tile([128, N], F16, tag=f"qT{p}", name=f"qT{p}") for p in range(2)]
            kT = [persist.tile([128, N], F16, tag=f"kT{p}", name=f"kT{p}") for p in range(2)]
            ctxT = [persist.tile([128, N], F16, tag=f"ctxT{p}", name=f"ctxT{p}") for p in range(2)]
            # V for 4 local heads: [nk-part, t, head*65 + (0:64 | ones)]
            v_all = persist.tile([128, NT, 4 * (HD + 1)], F16, tag="v", name="v")
            for h in range(4):
                nc.vector.memset(v_all[:, :, h * 65 + 64], 1.0)

            def x_dma(n):
                # n=0,2: per-k on the (startup-idle) GpSimd queue so the
                # first K-projection groups start on partial arrivals;
                # n=1,3: one monolithic 3D DMA each on the Sync queue,
                # pipelined behind the weights.
                if n % 2 == 0:
                    for k in range(KC):
                        nc.gpsimd.dma_start(
                            out=x_all[:, k, n, :],
                            in_=xT[k * 128:(k + 1) * 128, n * NQ:(n + 1) * NQ])
                else:
                    nc.sync.dma_start(
                        out=x_all[:, :, n, :],
                        in_=xT.rearrange("(k p) t -> p k t", p=128)[
                            :, :, n * NQ:(n + 1) * NQ])

            # ---- projection parcels ----
            # kq_group(p, n): 8 accumulating MMs + bias-add, emitted as
            # 4x(2 MMs) + 1 DVE parcel so each slots into per-t ACT slack.
            _pp = {}

            def kq_mms(p, n, wsb, key, ks):
                if ks == 0:
                    _pp[key] = ppool.tile([128, NQ], F32, tag="pp", name="pp")
                ps = _pp[key]
                for k in (ks, ks + 1):
                    nc.tensor.matmul(
                        ps, wsb[:, k, p * 128:(p + 1) * 128],
                        x_all[:, k, n, :], start=(k == 0), stop=(k == KC - 1))

            def kq_fin(p, n, bsb, dst, key):
                ps = _pp.pop(key)
                nc.vector.tensor_tensor(
                    out=dst[p][:, n * NQ:(n + 1) * NQ], in0=ps,
                    in1=bsb[:, p:p + 1].broadcast_to([128, NQ]), op=add)

            def kq_parcels(p, n, wsb, bsb, dst, key):
                out = [lambda ks=ks: kq_mms(p, n, wsb, key, ks)
                       for ks in (0, 2, 4, 6)]
                out.append(lambda: kq_fin(p, n, bsb, dst, key))
                return out

            def kq_pair_mms(p, n0, n1, wsb, key, ks):
                # two n-chunks accumulate in parallel pp banks; each weight
                # chunk is loaded once and streams both moving operands.
                if ks == 0:
                    _pp[(key, 0)] = ppool.tile([128, NQ], F32, tag="pp",
                                               name="pp")
                    _pp[(key, 1)] = ppool.tile([128, NQ], F32, tag="pp",
                                               name="pp")
                for k in (ks, ks + 1):
                    for i, n in enumerate((n0, n1)):
                        nc.tensor.matmul(
                            _pp[(key, i)], wsb[:, k, p * 128:(p + 1) * 128],
                            x_all[:, k, n, :], start=(k == 0),
                            stop=(k == KC - 1))

            def kq_pair_parcels(p, n0, n1, wsb, bsb, dst, key):
                out = [lambda ks=ks: kq_pair_mms(p, n0, n1, wsb, key, ks)
                       for ks in (0, 2, 4, 6)]

                def fin():
                    for i, n in enumerate((n0, n1)):
                        ps = _pp.pop((key, i))
                        nc.vector.tensor_tensor(
                            out=dst[p][:, n * NQ:(n + 1) * NQ], in0=ps,
                            in1=bsb[:, p:p + 1].broadcast_to([128, NQ]),
                            op=add)
                out.append(fin)
                return out

            def v_mms(n, tl, key, ks):
                if ks == 0:
                    _pp[key] = ppool.tile([128, NQ], F32, tag="pp", name="pp")
                ps = _pp[key]
                for k in (ks, ks + 1):
                    nc.tensor.matmul(
                        ps[:, 0:256], x_all[:, k, n, tl * 128:(tl + 1) * 128],
                        wv_sb[:, k, :], start=(k == 0), stop=(k == KC - 1))

            def v_fin(n, tl, key):
                ps = _pp.pop(key)
                t = n * 4 + tl
                vv = v_all[:, t, :].rearrange("p (h c) -> p h c", c=65)
                nc.vector.tensor_copy(
                    out=vv[:, :, 0:64],
                    in_=ps[:, 0:256].rearrange("p (h c) -> p h c", c=64))

            def v_parcels(n, tl):
                key = ("v", n, tl)
                out = [lambda ks=ks: v_mms(n, tl, key, ks)
                       for ks in (0, 2, 4, 6)]
                out.append(lambda: v_fin(n, tl, key))
                return out

            # ---- out-projection parcels ----
            o_tiles = {}

            def out_proj_parcel(kp, m):
                # both eo halves in one parcel: the stationary ctxT chunk is
                # loaded once and serves two 512-wide matmuls.
                if kp == 0:
                    o_tiles[m] = opool.tile([128, EMB], F32, tag="o", name="o")
                o = o_tiles[m]
                pos = []
                for eo in range(2):
                    po = ppool.tile([128, NQ], F32, tag="pp", name="pp")
                    nc.tensor.matmul(
                        po, ctxT[kp][:, m * 128:(m + 1) * 128],
                        wo_sb[:, kp, eo * NQ:(eo + 1) * NQ],
                        start=True, stop=True)
                    pos.append(po)
                for eo in range(2):
                    if kp == 0:
                        nc.vector.tensor_copy(o[:, eo * NQ:(eo + 1) * NQ],
                                              pos[eo])
                    else:
                        o16 = o_tiles[("f", m)]
                        nc.vector.tensor_tensor(
                            out=o16[:, eo * NQ:(eo + 1) * NQ],
                            in0=o[:, eo * NQ:(eo + 1) * NQ], in1=pos[eo],
                            op=add)

            def out_proj_p1(m):
                o_tiles[("f", m)] = o16pool.tile([128, EMB], F16, tag="o16",
                                                 name="o16")
                out_proj_parcel(1, m)
                o16 = o_tiles.pop(("f", m))
                o_tiles.pop(m)
                for eo in range(2):
                    nc.sync.dma_start(
                        out=out_part[m * 128:(m + 1) * 128,
                                     eo * NQ:(eo + 1) * NQ],
                        in_=o16[:, eo * NQ:(eo + 1) * NQ])

            # ---- attention machinery ----
            def s_pair(p, j, t):
                sg = spool.tile([128, 2, NQ], F32, tag="s", name="s")
                for h in range(2):
                    nc.tensor.matmul(
                        sg[:, h, :],
                        kT[p][64 * h:64 * h + 64, t * 128:(t + 1) * 128],
                        qT[p][64 * h:64 * h + 64, j * NQ:(j + 1) * NQ],
                        start=True, stop=True)
                return sg

            def exp_act(sg):
                e = epool.tile([128, 2, NQ], F16, tag="e", name="e")
                nc.scalar.activation(e, sg, FP.Exp, scale=SCALE)
                return e

            def ctx_pair(p, cps, e, t):
                for h in range(2):
                    hl = 2 * p + h
                    nc.tensor.matmul(
                        cps[h], v_all[:, t, hl * 65:(hl + 1) * 65],
                        e[:, h, :], start=(t == 0), stop=(t == NT - 1))

            def normalize(p, j, h, cps):
                # copy out of PSUM first (frees the ctx bank for next j);
                # rowsum sits in row 64 -> stage to partition 0 for the
                # gpsimd broadcast, reciprocal on the [1, 512] staged row.
                cs = npool.tile([65, NQ], F32, tag="cs", name="cs")
                nc.vector.tensor_copy(cs, cps[h])
                rs = npool.tile([1, NQ], F32, tag="rs", name="rs")
                nc.vector.tensor_copy(rs, cs[64:65, :])
                rc = npool.tile([1, NQ], F32, tag="rc", name="rc")
                nc.vector.reciprocal_approx_fast(rc, rs)
                rb = npool.tile([64, NQ], F32, tag="rb", name="rb")
                nc.gpsimd.partition_broadcast(rb, rc)
                nc.vector.scalar_tensor_tensor(
                    out=ctxT[p][h * 64:(h + 1) * 64, j * NQ:(j + 1) * NQ],
                    in0=cs[0:64, :], scalar=1.0, in1=rb,
                    op0=mult, op1=mult)

            # ---- schedule ----
            # One flat software pipeline over all 128 (p, j, t) iterations:
            # iteration g emits S-pair(g), exp(g), then ctx-pair(g-2) (lag 2
            # so ctx never waits on the exp semaphore), then filler parcels
            # from sched[g]. j/p boundaries are crossed seamlessly; the
            # normalize for j is emitted right after its last ctx pair.
            sched = {}

            def put(g, f):
                sched.setdefault(g, []).append(f)

            cps_by = {}
            pend = []

            def emit_ctx(g2):
                p2, r2 = divmod(g2, 64)
                j2, t2 = divmod(r2, 16)
                if t2 == 0:
                    cps_by[(p2, j2)] = [
                        cpool.tile([65, NQ], F32, tag=f"c{h}", name=f"c{h}")
                        for h in range(2)]
                e2 = pend.pop(0)
                ctx_pair(p2, cps_by[(p2, j2)], e2, t2)
                if t2 == NT - 1:
                    cps2 = cps_by.pop((p2, j2))
                    for h in range(2):
                        normalize(p2, j2, h, cps2)

            # ---------- prefix ----------
            # Sync queue: [biases, wk, wv, wq, x1, x3]; GpSimd queue: x0, x2
            # per-k. The first K group starts at wk + x0[0]; Q0(0) (the first
            # S dependency) completes right after wq lands.
            nc.sync.dma_start(out=bk_sb, in_=bkd.rearrange("(m p) -> p m", p=128))
            nc.sync.dma_start(out=bq_sb, in_=bqd.rearrange("(m p) -> p m", p=128))
            # pair-0 halves of wk/wq first: Q0(0) (the first S dependency)
            # completes as soon as wq[:,:,0:128] lands; pair-1 halves are
            # deferred into the filler schedule (needed from slot ~17).
            nc.sync.dma_start(out=wk_sb[:, :, 0:128], in_=wkT.rearrange(
                "(k p) d -> p k d", p=128)[:, :, 0:128])
            nc.sync.dma_start(out=wq_sb[:, :, 0:128], in_=wqT.rearrange(
                "(k p) d -> p k d", p=128)[:, :, 0:128])
            x_dma(0)
            x_dma(2)
            nc.sync.dma_start(out=x_all[:, :, 1, :],
                              in_=xT.rearrange("(k p) t -> p k t", p=128)[
                                  :, :, NQ:2 * NQ])
            nc.sync.dma_start(out=wv_sb, in_=wvT.rearrange(
                "(k p) d -> p k d", p=128))
            nc.sync.dma_start(out=x_all[:, :, 3, :],
                              in_=xT.rearrange("(k p) t -> p k t", p=128)[
                                  :, :, 3 * NQ:4 * NQ])
            for f in kq_parcels(0, 0, wk_sb, bk_sb, kT, ("k0", 0)):
                f()
            for f in kq_parcels(0, 0, wq_sb, bq_sb, qT, ("q0", 0)):
                f()

            # ---------- filler schedule ----------
            # pair-0 window (g 0..63):
            #   V(n, tl) whole-group at g=4n+tl+1 (ctx(t) runs at g=t+2)
            #   K0(n) parcels at g=4n-4..4n-1; Q0(n) parcels at g=16n-5..
            for n in range(NJ):
                for tl in range(4):
                    for i, f in enumerate(v_parcels(n, tl)):
                        put(4 * n + tl + 2 + i, f)
            for n in (1, 2, 3):
                for i, f in enumerate(kq_parcels(0, n, wk_sb, bk_sb, kT,
                                                 ("k0", n))):
                    put(4 * n - 4 + min(i, 3), f)
                for i, f in enumerate(kq_parcels(0, n, wq_sb, bq_sb, qT,
                                                 ("q0", n))):
                    put(16 * n - 5 + i, f)
            put(6, lambda: nc.sync.dma_start(
                out=wk_sb[:, :, 128:256], in_=wkT.rearrange(
                    "(k p) d -> p k d", p=128)[:, :, 128:256]))
            put(8, lambda: nc.sync.dma_start(
                out=wq_sb[:, :, 128:256], in_=wqT.rearrange(
                    "(k p) d -> p k d", p=128)[:, :, 128:256]))
            put(13, lambda: nc.sync.dma_start(
                out=wo_sb, in_=woT.rearrange("(k p) e -> p k e", p=128)))
            free_iters = [g for g in range(20, 64)
                          if len(sched.get(g, [])) == 0]
            fill = []
            fill += kq_pair_parcels(1, 0, 1, wk_sb, bk_sb, kT, ("k1", 0))
            fill += kq_pair_parcels(1, 0, 1, wq_sb, bq_sb, qT, ("q1", 0))
            fill += kq_pair_parcels(1, 2, 3, wk_sb, bk_sb, kT, ("k1", 2))
            fill += kq_pair_parcels(1, 2, 3, wq_sb, bq_sb, qT, ("q1", 2))
            for i, f in enumerate(fill):
                put(free_iters[i * len(free_iters) // len(fill)], f)
            # pair-1 window (g 64..127): out-proj pass 0 at g=64..95;
            # pass 1 for j at g=64+16(j+1)+2.. (8 parcels); j=3 in the tail.
            for m in range(NT):
                put(64 + 2 * m, lambda m=m: out_proj_parcel(0, m))
            for j in range(NJ - 1):
                for i, m in enumerate(range(4 * j, 4 * j + 4)):
                    put(64 + 16 * (j + 1) + 2 + 3 * i,
                        lambda m=m: out_proj_p1(m))

            # ---------- main pipeline (batches of 2 t-iterations) ----------
            # [S-pair(g), S-pair(g+1)] then [ctx(g-2), ctx(g-1)] so the PE
            # switches between 64-row-tile mode and full-array mode once per
            # batch instead of twice per iteration (each switch exposes the
            # ~150ns pipe drain to the next matmul).
            # ctx lag starts at 8 (j=0's V projections spread over the
            # first ~24 slots) and tightens to 2 by mid-j=1 so the drain
            # after the last exp stays short.
            next_g2 = 0
            for gb in range(0, 128, 2):
                sgs = []
                for g in (gb, gb + 1):
                    p, r = divmod(g, 64)
                    j, t = divmod(r, 16)
                    sgs.append(s_pair(p, j, t))
                for sg in sgs:
                    pend.append(exp_act(sg))
                for g in (gb, gb + 1):
                    for f in sched.get(g, ()):
                        f()
                lag = 8 if gb < 32 else max(2, 8 - (gb - 32) // 4)
                while len(pend) > lag:
                    emit_ctx(next_g2)
                    next_g2 += 1

            # ---------- tail: ctx/normalize for (p1, j3, t14..15) ----------
            # then pipelined per-m normalize-chunk + out-proj + DMA.
            p3, j3 = 1, NJ - 1
            cps3 = None
            last_e = None
            for g2 in (126, 127):
                t2 = g2 % 16
                e2 = pend.pop(0)
                ctx_pair(p3, cps_by[(p3, j3)], e2, t2)
                last_e = e2
            cps3 = cps_by.pop((p3, j3))
            css, rbs = [], []
            for h in range(2):
                cs = npool.tile([65, NQ], F32, tag="cs", name="cs")
                nc.vector.tensor_copy(cs, cps3[h])
                rs = npool.tile([1, NQ], F32, tag="rs", name="rs")
                nc.vector.tensor_copy(rs, cs[64:65, :])
                rc = npool.tile([1, NQ], F32, tag="rc", name="rc")
                nc.vector.reciprocal_approx_fast(rc, rs)
                rb = npool.tile([64, NQ], F32, tag="rb", name="rb")
                nc.gpsimd.partition_broadcast(rb, rc)
                css.append(cs)
                rbs.append(rb)
            for m in range(4 * j3, 4 * j3 + 4):
                mo = (m - 4 * j3) * 128
                for h in range(2):
                    nc.vector.scalar_tensor_tensor(
                        out=ctxT[p3][h * 64:(h + 1) * 64,
                                     j3 * NQ + mo:j3 * NQ + mo + 128],
                        in0=css[h][0:64, mo:mo + 128], scalar=1.0,
                        in1=rbs[h][:, mo:mo + 128], op0=mult, op1=mult)
                out_proj_p1(m)

    nc.compile()
    return nc


_NC_CACHE = {}


def _get_program():
    if "nc" not in _NC_CACHE:
        _NC_CACHE["nc"] = build_program()
    return _NC_CACHE["nc"]


def make_in_maps(x, Wq, bq, Wk, bk, Wv, bv, Wo, bo):
    x = np.asarray(x)
    xTs = [np.ascontiguousarray(x[b].T.astype(np.float16)) for b in range(B)]
    in_maps = []
    for c in range(N_CORES):
        b, hg = divmod(c, TP)
        sl = slice(hg * DQ, (hg + 1) * DQ)
        in_maps.append({
            "xT": xTs[b],
            "wqT": np.ascontiguousarray(np.asarray(Wq, np.float16)[sl, :].T),
            "wkT": np.ascontiguousarray(np.asarray(Wk, np.float16)[sl, :].T),
            "wvT": np.ascontiguousarray(np.asarray(Wv, np.float16)[sl, :].T),
            "woT": np.ascontiguousarray(np.asarray(Wo, np.float16)[:, sl].T),
            "bq_s": np.ascontiguousarray(np.asarray(bq, np.float32)[sl]),
            "bk_s": np.ascontiguousarray(np.asarray(bk, np.float32)[sl]),
        })
    return in_maps


def assemble_output(results, Wv_bias_term):
    out = np.empty((B, N, EMB), np.float32)
    for b in range(B):
        acc = results[b * TP]["out_part"].astype(np.float32)
        for g in range(1, TP):
            acc = acc + results[b * TP + g]["out_part"].astype(np.float32)
        out[b] = acc + Wv_bias_term
    return out


def kernel(x, Wq, bq, Wk, bk, Wv, bv, Wo, bo):
    nc = _get_program()
    in_maps = make_in_maps(x, Wq, bq, Wk, bk, Wv, bv, Wo, bo)
    res = run_bass_kernel_spmd(nc, in_maps, list(range(N_CORES)))
    bias_term = (np.asarray(bv, np.float32) @ np.asarray(Wo, np.float32).T
                 + np.asarray(bo, np.float32))
    return assemble_output(res.results, bias_term)
